# revision 1
# baseline (speedup 1.0000x reference)
"""Bahdanau additive attention on 8 Trainium2 NeuronCores (Bass/Tile).

Reference computation (per batch b):
    wq   = query @ wa_w.T + wa_b                      # [1, H]
    uk   = keys  @ ua_w.T + ua_b                      # [L, H]
    s    = tanh(wq + uk) @ va_w.T + va_b              # [L]
    s    = where(mask, -inf, s)
    w    = softmax(s)                                 # [L]
    ctx  = w @ keys                                   # [1, H]

Sharding: data-parallel over batch B=32 -> 4 batches per core; small
weights replicated.  The heavy matmul runs on the PE in float32r
(full-rate fp32, tf32-like operand rounding).

Device-side structure (per core, BC=4 batches):
  - keys arrive pre-transposed (host) as keysT [H, L]; the big matmul
    computes uk^T [k, l] so the per-batch wq[k]+wa_b[k]+ua_b[k] (tiny,
    host-precomputed) is a per-partition ACT bias fused into the tanh.
  - scores = va . tanh(.) is a PE matmul with va as a [128,1] stationary.
  - softmax uses a FIXED per-batch max (from the first 512-l tile).  This
    is safe here: |scores| <= ||va||_1 (tanh in [-1,1]), far below fp32
    exp range, so no running-max rescaling is needed.  exp + per-tile sum
    fuse into one ACT op (accum_out); softmax shift invariance drops va_b.
  - the weighted key sum ctx^T = sum_l e_l * keysT[:, l] runs on the DVE
    (scalar_tensor_tensor multiply with accum_out) against the SAME keysT
    tiles pass 1 just consumed -> keys are read from HBM exactly once.
    exp weights are partition-broadcast via a tiny PE ones-matmul.
  - per-batch outputs are the unnormalized ctx^T [128, HC] and the 4
    per-tile exp sums; the host divides by their total and transposes
    during the gather/unshard step (a 32 KB epilogue).
"""

import os
import numpy as np
from contextlib import ExitStack

import concourse.bass as bass  # noqa: F401
import concourse.bacc as bacc
import concourse.tile as tile
from concourse import mybir
from concourse.bass_utils import run_bass_kernel_spmd

B, L, H = 32, 2048, 1024
NCORES = 8
BC = B // NCORES          # batches per core
HC = H // 128             # 128-chunks of the hidden dim
LT = 512                  # l-tile width
NLT = L // LT
NSLOT = 8                 # slot-dim padding (last batch uses NLT+1 slots)

F32 = mybir.dt.float32
F32R = mybir.dt.float32r
AF = mybir.ActivationFunctionType
AX = mybir.AxisListType
OP = mybir.AluOpType

_nc = None
LAST_RESULT = None


def _body(nc, tc, ctx, d):
    consts = ctx.enter_context(tc.tile_pool(name="consts", bufs=1))
    kpool = ctx.enter_context(tc.tile_pool(name="kT", bufs=6))
    tpool = ctx.enter_context(tc.tile_pool(name="tk", bufs=12))
    small = ctx.enter_context(tc.tile_pool(name="small", bufs=2))
    p_uk = ctx.enter_context(tc.tile_pool(name="p_uk", bufs=5, space="PSUM"))
    p_sc = ctx.enter_context(tc.tile_pool(name="p_sc", bufs=2, space="PSUM"))
    p_wb = ctx.enter_context(tc.tile_pool(name="p_wb", bufs=1, space="PSUM"))

    # ---- constants / weights on the ACT HWDGE queue so the keysT stream
    # (sync queue) is not delayed behind them ----
    biasT = consts.tile([128, HC * BC], F32)
    nc.scalar.dma_start(biasT[:], d["biasT"])
    vaT = consts.tile([128, HC], F32R)
    nc.scalar.dma_start(vaT[:], d["vaT"].bitcast(F32R))
    ones_r = consts.tile([1, 128], F32R)
    nc.scalar.dma_start(ones_r[:], d["ones"].bitcast(F32R))
    uawT = consts.tile([128, HC, H], F32R)
    for hc in range(HC):
        nc.scalar.dma_start(uawT[:, hc, :],
                            d["uawT"][:, hc * H : (hc + 1) * H].bitcast(F32R))

    # Per-batch state, created lazily inside the flat tile loop.
    bstate = {}

    def batch_state(b):
        if b not in bstate:
            mb = small.tile([1, L], F32, tag="mb")
            nc.sync.dma_start(mb[:], d["maskb"][b : b + 1, :])
            s_all = small.tile([1, NSLOT], F32, tag="s_all", name=f"s_all_{b}")
            pp_all = small.tile([128, HC, NSLOT], F32, tag="pp_all",
                                name=f"pp_all_{b}")
            bstate[b] = {
                "mb": mb,
                "negm0": None,
                "s_all": s_all,
                "pp_all": pp_all,
            }
        return bstate[b]

    def emit_scores(rec, kc):
        """One deferred score matmul for tile rec at chunk kc (its tanh is
        a full tile old, so this never stalls the PE)."""
        nc.tensor.matmul(
            rec["ps"][:], vaT[:, kc : kc + 1], rec["tks"][kc][:],
            start=(kc == 0), stop=(kc == HC - 1),
        )

    def emit_softmax(rec):
        """Mask add + exp(+sum) for tile rec; DVE/ACT only."""
        b, lt, l0, lw = rec["b"], rec["lt"], rec["l0"], rec["lw"]
        st = bstate[b]
        sm = small.tile([1, lw], F32, tag="sm")
        nc.vector.tensor_add(sm[:], rec["ps"][:],
                             st["mb"][0:1, l0 : l0 + lw])
        if lt == 0:
            mx = small.tile([1, 1], F32, tag="mx")
            nc.vector.tensor_reduce(mx[:], sm[:], axis=AX.X, op=OP.max)
            st["negm0"] = small.tile([1, 1], F32, tag="negm",
                                     name=f"negm0_{b}")
            nc.vector.tensor_scalar_mul(st["negm0"][:], mx[:], -1.0)
        e = small.tile([1, lw], F32R, tag="e", bufs=3, name=f"e_{b}_{lt}")
        nc.scalar.activation(e[:], sm[:], AF.Exp, bias=st["negm0"][0:1, 0:1],
                             scale=1.0,
                             accum_out=st["s_all"][0:1, rec["slot"] : rec["slot"] + 1])
        rec["e"] = e

    def emit_wsum(rec):
        """Weight broadcast (PE, input long ready) + DVE weighted key sum."""
        b, lt, lw = rec["b"], rec["lt"], rec["lw"]
        st = bstate[b]
        wb = p_wb.tile([128, lw], F32, tag="wb")
        nc.tensor.matmul(wb[:], ones_r[:], rec["e"][:], start=True, stop=True)
        for hc in range(HC):
            dump = small.tile([128, lw], F32, tag="dump")
            nc.vector.scalar_tensor_tensor(
                dump[:],
                rec["kT"][:, hc, :].bitcast(F32),
                1.0,
                wb[:],
                op0=OP.mult,
                op1=OP.mult,
                accum_out=st["pp_all"][:, hc, rec["slot"] : rec["slot"] + 1],
            )
        if rec["last"]:
            ns = rec["slot"] + 1
            acc = small.tile([128, HC], F32, tag="acc")
            nc.vector.tensor_reduce(acc[:], st["pp_all"][:, :, 0:ns],
                                    axis=AX.X, op=OP.add)
            nc.sync.dma_start(d["accout"][b, :, :], acc[:])
            nc.sync.dma_start(d["sout"][b : b + 1, 0:ns], st["s_all"][0:1, 0:ns])

    # tile plan: (b, l0, lw); the final batch splits its last 512 into 2x256
    plan = []
    for b in range(BC):
        widths = [LT] * NLT if b < BC - 1 else [LT] * (NLT - 1) + [LT // 2] * 2
        l0 = 0
        for i, w in enumerate(widths):
            plan.append({"b": b, "lt": i, "slot": i, "l0": l0, "lw": w,
                         "last": i == len(widths) - 1})
            l0 += w

    tiles = []
    for t, rec in enumerate(plan):
        b, l0, lw = rec["b"], rec["l0"], rec["lw"]
        batch_state(b)
        kT = kpool.tile([128, HC, lw], F32R, tag="kT")
        for hc in range(HC):
            nc.sync.dma_start(
                kT[:, hc, :],
                d["keysT"][b, hc * 128 : (hc + 1) * 128,
                           l0 : l0 + lw].bitcast(F32R),
            )
        ps = p_sc.tile([1, lw], F32, tag="ps")
        rec.update({"kT": kT, "tks": [], "ps": ps})

        def uk_mm(kc, hcs, pu):
            for hc in hcs:
                nc.tensor.matmul(
                    pu[:],
                    uawT[:, hc, kc * 128 : (kc + 1) * 128],
                    kT[:, hc, :],
                    start=(hc == 0),
                    stop=(hc == HC - 1),
                )

        pus = []
        for kc in range(HC):
            pu = p_uk.tile([128, lw], F32, tag="pu")
            pus.append(pu)
            if t == 0:
                # warm-up: consume only the first half of the weight/key
                # chunks so compute starts after ~3MB of DMA, not 6MB
                if kc % 2 == 0:
                    uk_mm(kc, range(HC // 2), pu)
                    continue
                uk_mm(kc, range(HC // 2), pu)
                uk_mm(kc - 1, range(HC // 2, HC), pus[kc - 1])
                uk_mm(kc, range(HC // 2, HC), pu)
            else:
                uk_mm(kc, range(HC), pu)
            for c in ([kc - 1, kc] if t == 0 else [kc]):
                tk = tpool.tile([128, lw], F32R, tag="tk")
                nc.scalar.activation(
                    tk[:], pus[c][:], AF.Tanh,
                    bias=biasT[:, c * BC + b : c * BC + b + 1], scale=1.0,
                )
                rec["tks"].append(tk)
            if t >= 1:
                prev = tiles[t - 1]
                if kc < 4:
                    emit_scores(prev, 2 * kc)
                    emit_scores(prev, 2 * kc + 1)
                elif kc == 4:
                    emit_softmax(prev)
                elif kc == HC - 1:
                    emit_wsum(prev)
            if t == len(plan) - 1 and kc >= 1:
                # final tile: its own scores lag one kc slot so the flush
                # chain is only tanh(7) -> sc(7) -> exp -> wsum
                emit_scores(rec, kc - 1)
        tiles.append(rec)

    # flush the final tile
    last = tiles[-1]
    emit_scores(last, HC - 1)
    emit_softmax(last)
    emit_wsum(last)

def build():
    nc = bacc.Bacc("TRN2", target_bir_lowering=False, debug=False,
                   num_devices=NCORES)
    d = {
        "keysT": nc.dram_tensor("keysT", [BC, H, L], F32, kind="ExternalInput").ap(),
        "uawT": nc.dram_tensor("uawT", [128, HC * H], F32, kind="ExternalInput").ap(),
        "vaT": nc.dram_tensor("vaT", [128, HC], F32, kind="ExternalInput").ap(),
        "biasT": nc.dram_tensor("biasT", [128, HC * BC], F32, kind="ExternalInput").ap(),
        "ones": nc.dram_tensor("ones", [1, 128], F32, kind="ExternalInput").ap(),
        "maskb": nc.dram_tensor("maskb", [BC, L], F32, kind="ExternalInput").ap(),
        "accout": nc.dram_tensor("accout", [BC, 128, HC], F32, kind="ExternalOutput").ap(),
        "sout": nc.dram_tensor("sout", [BC, NSLOT], F32, kind="ExternalOutput").ap(),
    }
    with tile.TileContext(nc) as tc, ExitStack() as ctx:
        _body(nc, tc, ctx, d)
    nc.compile()
    return nc


def _maybe_install_profile_hook():
    """BASS_TRACE=1 profiling under axon needs antenv.axon_hooks, which this
    image lacks; shim it with an in-memory module wired to libaxon_pjrt."""
    import sys, types
    if "antenv.axon_hooks" in sys.modules:
        return
    mod = types.ModuleType("antenv.axon_hooks")
    holder = [None]
    mod.set_axon_ntff_profile_hook = lambda h: holder.__setitem__(0, h)
    mod.get_axon_ntff_profile_hook = lambda: holder[0]
    sys.modules["antenv.axon_hooks"] = mod
    try:
        from trn_agent_boot.trn_boot import _ntff_profile_via_ctypes
        mod.set_axon_ntff_profile_hook(
            _ntff_profile_via_ctypes("/opt/axon/libaxon_pjrt.so"))
    except Exception:
        pass


def make_in_maps(query, keys, mask, wa_w, wa_b, ua_w, ua_b, va_w, va_b):
    query = np.asarray(query, dtype=np.float32)
    keys = np.asarray(keys, dtype=np.float32)
    mask = np.asarray(mask)
    wa_w = np.asarray(wa_w, dtype=np.float32)
    wa_b = np.asarray(wa_b, dtype=np.float32)
    ua_b = np.asarray(ua_b, dtype=np.float32)
    ua_w = np.asarray(ua_w, dtype=np.float32)
    va_w = np.asarray(va_w, dtype=np.float32)

    # lhsT chunk layout: arr[p, hc*H + k] = W[k, hc*128 + p]
    uawT = np.ascontiguousarray(
        ua_w.T.reshape(HC, 128, H).transpose(1, 0, 2).reshape(128, HC * H))
    vaT = np.ascontiguousarray(va_w[0].reshape(HC, 128).T)
    maskb = np.where(mask, np.float32(-1e30), np.float32(0.0)).astype(np.float32)
    keysT = np.ascontiguousarray(keys.transpose(0, 2, 1))  # [B, H, L]
    # wq + wa_b + ua_b on host (0.05% of the FLOPs)
    wq = query[:, 0, :] @ wa_w.T + wa_b + ua_b  # [B, H]

    in_maps = []
    for c in range(NCORES):
        bs = slice(c * BC, (c + 1) * BC)
        biasT = np.ascontiguousarray(
            wq[bs].T.reshape(HC, 128, BC).transpose(1, 0, 2).reshape(128, HC * BC))
        in_maps.append({
            "keysT": keysT[bs],
            "uawT": uawT,
            "vaT": vaT,
            "biasT": biasT,
            "ones": np.ones((1, 128), dtype=np.float32),
            "maskb": np.ascontiguousarray(maskb[bs]),
        })
    return in_maps


def kernel(query, keys, mask, wa_w, wa_b, ua_w, ua_b, va_w, va_b):
    global _nc, LAST_RESULT
    if os.environ.get("BASS_TRACE"):
        _maybe_install_profile_hook()
    if _nc is None:
        _nc = build()
    in_maps = make_in_maps(query, keys, mask, wa_w, wa_b, ua_w, ua_b, va_w, va_b)
    res = run_bass_kernel_spmd(_nc, in_maps, list(range(NCORES)))
    LAST_RESULT = res
    outs = []
    for c in range(NCORES):
        acc = res.results[c]["accout"]          # [BC, 128, HC] = ctx^T unnormalized
        sout = res.results[c]["sout"]  # [BC, NSLOT]; batch i uses NLT(+1) slots
        nslots = np.array([NLT + 1 if i == BC - 1 else NLT for i in range(BC)])
        ssum = np.array([sout[i, : nslots[i]].sum() for i in range(BC)])
        # ctx[b, hc*128+p] = acc[b, p, hc] / ssum[b]
        ctx = acc.transpose(0, 2, 1).reshape(BC, H) / ssum[:, None]
        outs.append(ctx)
    out = np.concatenate(outs, axis=0)
    return np.ascontiguousarray(out[:, None, :].astype(np.float32))



# revision 4
# speedup vs baseline: 1.0491x; 1.0491x over previous
"""Bahdanau additive attention on 8 Trainium2 NeuronCores (Bass/Tile).

Reference computation (per batch b):
    wq   = query @ wa_w.T + wa_b                      # [1, H]
    uk   = keys  @ ua_w.T + ua_b                      # [L, H]
    s    = tanh(wq + uk) @ va_w.T + va_b              # [L]
    s    = where(mask, -inf, s)
    w    = softmax(s)                                 # [L]
    ctx  = w @ keys                                   # [1, H]

Sharding: data-parallel over batch B=32 -> 4 batches per core; small
weights replicated.  The heavy matmul runs on the PE in bf16 (same
78.6 TF/s PE rate as fp32r, but half the DMA/SBUF footprint and 4x
faster weight loads via FWL; accuracy ~1e-3 rel, well inside 2e-2).

Device-side structure (per core, BC=4 batches):
  - keys arrive pre-transposed (host) as keysTr [128, HC, L] per batch;
    the big matmul computes uk^T [k, l] so the per-batch
    wq[k]+wa_b[k]+ua_b[k] (tiny, host-precomputed) is a per-partition
    ACT bias fused into the tanh.
  - scores = va . tanh(.) is a PE matmul with va as a [128,1] stationary.
  - softmax needs NO max subtraction: |scores| <= ||va||_1 ~ 26 << 88,
    so exp never overflows fp32.  exp + per-tile sum fuse into one ACT
    op (accum_out); softmax shift invariance drops va_b.
  - the weighted key sum ctx^T = sum_l e_l * keysT[:, l] runs on the DVE
    (scalar_tensor_tensor multiply with accum_out) against the SAME
    keysTr tiles pass 1 just consumed -> keys are read from HBM once.
    exp weights are partition-broadcast via a tiny PE ones-matmul.
  - tile 0 consumes weights hc-major (4 open PSUM accumulations) so the
    first matmul needs only uawT[hc0]+kT0[hc0] (~384KB) instead of the
    whole weight set; the PE starts ~1.5us in and HAM warms early.
  - per-batch outputs are the unnormalized ctx^T [128, HC] and the
    per-tile exp sums; the host divides by their total and transposes
    during the gather/unshard step (a 32 KB epilogue).
"""

import os
import numpy as np
from contextlib import ExitStack

import ml_dtypes

import concourse.bass as bass  # noqa: F401
import concourse.bacc as bacc
import concourse.tile as tile
from concourse import mybir
from concourse.bass_utils import run_bass_kernel_spmd

B, L, H = 32, 2048, 1024
NCORES = 8
BC = B // NCORES          # batches per core
HC = H // 128             # 128-chunks of the hidden dim
NSLOT = 8                 # slot-dim padding (last batch uses 5 slots)

# l-tile widths per batch; last batch ends with a small tile so the
# serial flush chain after the final matmul is short.
WIDTHS = [[512, 512, 512, 512]] * (BC - 1) + [[512, 512, 512, 384, 128]]

F32 = mybir.dt.float32
BF = mybir.dt.bfloat16
AF = mybir.ActivationFunctionType
AX = mybir.AxisListType
OP = mybir.AluOpType

_nc = None
LAST_RESULT = None


def _body(nc, tc, ctx, d):
    consts = ctx.enter_context(tc.tile_pool(name="consts", bufs=1))
    kpool = ctx.enter_context(tc.tile_pool(name="kT", bufs=8))
    tpool = ctx.enter_context(tc.tile_pool(name="tk", bufs=12))
    small = ctx.enter_context(tc.tile_pool(name="small", bufs=2))
    p_uk = ctx.enter_context(tc.tile_pool(name="p_uk", bufs=5, space="PSUM"))
    p_sc = ctx.enter_context(tc.tile_pool(name="p_sc", bufs=2, space="PSUM"))
    p_wb = ctx.enter_context(tc.tile_pool(name="p_wb", bufs=1, space="PSUM"))

    # ---- weights on the ACT HWDGE queue (keysTr stream owns the sync
    # queue).  uawT chunks go FIRST, in hc order, to feed the hc-major
    # tile-0 warmup; the small consts follow. ----
    uawT = consts.tile([128, HC, H], BF)
    for hc in range(HC):
        nc.scalar.dma_start(uawT[:, hc, :], d["uawT"][:, hc * H : (hc + 1) * H])
    biasT = consts.tile([128, HC * BC], F32)
    nc.scalar.dma_start(biasT[:], d["biasT"])
    vaT = consts.tile([128, HC], BF)
    nc.scalar.dma_start(vaT[:], d["vaT"])
    ones_r = consts.tile([1, 128], BF)
    nc.scalar.dma_start(ones_r[:], d["ones"])

    # Per-batch state, created lazily inside the flat tile loop.
    bstate = {}

    def batch_state(b):
        if b not in bstate:
            mb = small.tile([1, L], F32, tag="mb")
            nc.gpsimd.dma_start(mb[:], d["maskb"][b : b + 1, :])
            s_all = small.tile([1, NSLOT], F32, tag="s_all", name=f"s_all_{b}")
            pp_all = small.tile([128, HC, NSLOT], F32, tag="pp_all",
                                name=f"pp_all_{b}")
            bstate[b] = {"mb": mb, "s_all": s_all, "pp_all": pp_all}
        return bstate[b]

    def emit_scores(rec, kc):
        """One deferred score matmul for tile rec at chunk kc (its tanh is
        a full tile old, so this never stalls the PE)."""
        nc.tensor.matmul(
            rec["ps"][:], vaT[:, kc : kc + 1], rec["tks"][kc][:],
            start=(kc == 0), stop=(kc == HC - 1),
        )

    def emit_softmax(rec):
        """Mask add + exp(+sum) for tile rec; DVE/ACT only.  No max
        subtraction: scores are bounded by ||va||_1 << fp32 exp range."""
        b, l0, lw = rec["b"], rec["l0"], rec["lw"]
        st = bstate[b]
        sm = small.tile([1, lw], F32, tag="sm")
        nc.vector.tensor_add(sm[:], rec["ps"][:],
                             st["mb"][0:1, l0 : l0 + lw])
        e = small.tile([1, lw], BF, tag="e", bufs=3, name=f"e_{b}_{rec['lt']}")
        nc.scalar.activation(e[:], sm[:], AF.Exp, bias=0.0, scale=1.0,
                             accum_out=st["s_all"][0:1, rec["slot"] : rec["slot"] + 1])
        rec["e"] = e

    def emit_wbcast(rec):
        """Partition-broadcast of the exp weights: tiny PE ones-matmul,
        then an ACT copy out of PSUM into a bf16 SBUF tile."""
        lw = rec["lw"]
        wb = p_wb.tile([128, lw], F32, tag="wb")
        nc.tensor.matmul(wb[:], ones_r[:], rec["e"][:], start=True, stop=True)
        wbs = small.tile([128, lw], BF, tag="wbs", bufs=2,
                         name=f"wbs_{rec['b']}_{rec['lt']}")
        nc.scalar.activation(wbs[:], wb[:], AF.Copy)
        rec["wbs"] = wbs

    def emit_wsum(rec):
        """DVE weighted key sum against the resident keysTr tile."""
        b = rec["b"]
        st = bstate[b]
        for hc in range(HC):
            dump = small.tile([128, rec["lw"]], BF, tag="dump")
            nc.vector.scalar_tensor_tensor(
                dump[:],
                rec["kT"][:, hc, :],
                1.0,
                rec["wbs"][:],
                op0=OP.mult,
                op1=OP.mult,
                accum_out=st["pp_all"][:, hc, rec["slot"] : rec["slot"] + 1],
            )
        if rec["last"]:
            ns = rec["slot"] + 1
            acc = small.tile([128, HC], F32, tag="acc")
            nc.vector.tensor_reduce(acc[:], st["pp_all"][:, :, 0:ns],
                                    axis=AX.X, op=OP.add)
            nc.gpsimd.dma_start(d["accout"][b, :, :], acc[:])
            nc.gpsimd.dma_start(d["sout"][b : b + 1, 0:ns], st["s_all"][0:1, 0:ns])

    # tile plan: flat list of (b, l0, lw)
    plan = []
    for b in range(BC):
        l0 = 0
        for i, w in enumerate(WIDTHS[b]):
            plan.append({"b": b, "lt": i, "slot": i, "l0": l0, "lw": w,
                         "last": i == len(WIDTHS[b]) - 1})
            l0 += w

    tiles = []
    for t, rec in enumerate(plan):
        b, l0, lw = rec["b"], rec["l0"], rec["lw"]
        batch_state(b)
        kT = kpool.tile([128, HC, lw], BF, tag="kT")
        if t == 0:
            # per-hc arrival so the hc-major warmup starts after ~384KB
            for hc in range(HC):
                nc.sync.dma_start(kT[:, hc, :], d["keysTr"][b, :, hc, l0 : l0 + lw])
        else:
            nc.sync.dma_start(kT[:, :, :], d["keysTr"][b, :, :, l0 : l0 + lw])
        ps = p_sc.tile([1, lw], F32, tag="ps")
        rec.update({"kT": kT, "tks": [], "ps": ps})

        def mm(pu, kc, hc):
            nc.tensor.matmul(
                pu[:],
                uawT[:, hc, kc * 128 : (kc + 1) * 128],
                kT[:, hc, :],
                start=(hc == 0),
                stop=(hc == HC - 1),
            )

        def tanh(kc, pu):
            tk = tpool.tile([128, lw], BF, tag="tk")
            nc.scalar.activation(
                tk[:], pu[:], AF.Tanh,
                bias=biasT[:, kc * BC + b : kc * BC + b + 1], scale=1.0,
            )
            rec["tks"].append(tk)

        if t == 0:
            # warm-up: hc-major over kc 0..3 (4 open PSUM accumulations),
            # so compute starts as soon as uawT[hc0]+kT0[hc0] land.
            pus = [p_uk.tile([128, lw], F32, tag="pu", name=f"pu_w{kc}")
                   for kc in range(4)]
            for hc in range(HC):
                for kc in range(4):
                    mm(pus[kc], kc, hc)
            # pass B: kc 4..7 kc-major (weights all resident by now),
            # pipelined with pass A's tanhs.
            for kc in range(4, HC):
                pu = p_uk.tile([128, lw], F32, tag="pu")
                for hc in range(HC):
                    mm(pu, kc, hc)
                tanh(kc - 4, pus[kc - 4])
                pus.append(pu)
            for kc in range(4, HC):
                tanh(kc, pus[kc])
        else:
            for kc in range(HC):
                pu = p_uk.tile([128, lw], F32, tag="pu")
                for hc in range(HC):
                    mm(pu, kc, hc)
                tanh(kc, pu)
                prev = tiles[t - 1]
                if kc < 4:
                    emit_scores(prev, 2 * kc)
                    emit_scores(prev, 2 * kc + 1)
                elif kc == 4:
                    emit_softmax(prev)
                elif kc == 5:
                    emit_wbcast(prev)
                elif kc == HC - 1:
                    emit_wsum(prev)
                if t == len(plan) - 1 and kc >= 1:
                    # final tile: its own scores lag one kc slot so the
                    # flush chain is only tanh(7) -> sc(7) -> exp -> wsum
                    emit_scores(rec, kc - 1)
        tiles.append(rec)

    # flush the final tile
    last = tiles[-1]
    emit_scores(last, HC - 1)
    emit_softmax(last)
    emit_wbcast(last)
    emit_wsum(last)


def build():
    nc = bacc.Bacc("TRN2", target_bir_lowering=False, debug=False,
                   num_devices=NCORES)
    d = {
        "keysTr": nc.dram_tensor("keysTr", [BC, 128, HC, L], BF, kind="ExternalInput").ap(),
        "uawT": nc.dram_tensor("uawT", [128, HC * H], BF, kind="ExternalInput").ap(),
        "vaT": nc.dram_tensor("vaT", [128, HC], BF, kind="ExternalInput").ap(),
        "biasT": nc.dram_tensor("biasT", [128, HC * BC], F32, kind="ExternalInput").ap(),
        "ones": nc.dram_tensor("ones", [1, 128], BF, kind="ExternalInput").ap(),
        "maskb": nc.dram_tensor("maskb", [BC, L], F32, kind="ExternalInput").ap(),
        "accout": nc.dram_tensor("accout", [BC, 128, HC], F32, kind="ExternalOutput").ap(),
        "sout": nc.dram_tensor("sout", [BC, NSLOT], F32, kind="ExternalOutput").ap(),
    }
    with tile.TileContext(nc) as tc, ExitStack() as ctx:
        _body(nc, tc, ctx, d)
    nc.compile()
    return nc


def _maybe_install_profile_hook():
    """BASS_TRACE=1 profiling under axon needs antenv.axon_hooks, which this
    image lacks; shim it with an in-memory module wired to libaxon_pjrt."""
    import sys, types
    if "antenv.axon_hooks" in sys.modules:
        return
    mod = types.ModuleType("antenv.axon_hooks")
    holder = [None]
    mod.set_axon_ntff_profile_hook = lambda h: holder.__setitem__(0, h)
    mod.get_axon_ntff_profile_hook = lambda: holder[0]
    sys.modules["antenv.axon_hooks"] = mod
    try:
        from trn_agent_boot.trn_boot import _ntff_profile_via_ctypes
        mod.set_axon_ntff_profile_hook(
            _ntff_profile_via_ctypes("/opt/axon/libaxon_pjrt.so"))
    except Exception:
        pass


def make_in_maps(query, keys, mask, wa_w, wa_b, ua_w, ua_b, va_w, va_b):
    bf16 = ml_dtypes.bfloat16
    query = np.asarray(query, dtype=np.float32)
    keys = np.asarray(keys, dtype=np.float32)
    mask = np.asarray(mask)
    wa_w = np.asarray(wa_w, dtype=np.float32)
    wa_b = np.asarray(wa_b, dtype=np.float32)
    ua_b = np.asarray(ua_b, dtype=np.float32)
    ua_w = np.asarray(ua_w, dtype=np.float32)
    va_w = np.asarray(va_w, dtype=np.float32)

    # lhsT chunk layout: arr[p, hc*H + k] = W[k, hc*128 + p]
    uawT = np.ascontiguousarray(
        ua_w.T.reshape(HC, 128, H).transpose(1, 0, 2).reshape(128, HC * H)
    ).astype(bf16)
    vaT = np.ascontiguousarray(va_w[0].reshape(HC, 128).T).astype(bf16)
    maskb = np.where(mask, np.float32(-1e30), np.float32(0.0)).astype(np.float32)
    # keysTr[b, p, hc, l] = keys[b, l, hc*128+p]
    keysTr = np.ascontiguousarray(
        keys.transpose(0, 2, 1).reshape(B, HC, 128, L).transpose(0, 2, 1, 3)
    ).astype(bf16)
    # wq + wa_b + ua_b on host (0.05% of the FLOPs)
    wq = query[:, 0, :] @ wa_w.T + wa_b + ua_b  # [B, H]

    in_maps = []
    for c in range(NCORES):
        bs = slice(c * BC, (c + 1) * BC)
        biasT = np.ascontiguousarray(
            wq[bs].T.reshape(HC, 128, BC).transpose(1, 0, 2).reshape(128, HC * BC))
        in_maps.append({
            "keysTr": keysTr[bs],
            "uawT": uawT,
            "vaT": vaT,
            "biasT": biasT,
            "ones": np.ones((1, 128), dtype=bf16),
            "maskb": np.ascontiguousarray(maskb[bs]),
        })
    return in_maps


def kernel(query, keys, mask, wa_w, wa_b, ua_w, ua_b, va_w, va_b):
    global _nc, LAST_RESULT
    if os.environ.get("BASS_TRACE"):
        _maybe_install_profile_hook()
    if _nc is None:
        _nc = build()
    in_maps = make_in_maps(query, keys, mask, wa_w, wa_b, ua_w, ua_b, va_w, va_b)
    res = run_bass_kernel_spmd(_nc, in_maps, list(range(NCORES)))
    LAST_RESULT = res
    outs = []
    for c in range(NCORES):
        acc = res.results[c]["accout"]          # [BC, 128, HC] = ctx^T unnormalized
        sout = res.results[c]["sout"]  # [BC, NSLOT]; batch i uses len(WIDTHS[i]) slots
        ssum = np.array([sout[i, : len(WIDTHS[i])].sum() for i in range(BC)])
        # ctx[b, hc*128+p] = acc[b, p, hc] / ssum[b]
        ctx = acc.transpose(0, 2, 1).reshape(BC, H) / ssum[:, None]
        outs.append(ctx)
    out = np.concatenate(outs, axis=0)
    return np.ascontiguousarray(out[:, None, :].astype(np.float32))
